# revision 12
# baseline (speedup 1.0000x reference)
"""PositionalSparseLinear v6: affiliation-bucketed group pools, contiguous
pool DMA, warm PE.

out[b, o] = sum_k x[b, conn[o,k]] * w[o,k]  ==  out = x @ S  (S sparse).

Sharding: out_features across 8 cores (1024 outs each), x replicated.

Per core the 1024 outputs form 2 groups x 4 tiles(128). For each group the
~7100 distinct x-rows its tiles touch are bucketed by the exact 4-bit mask of
which tiles use them, and buckets are laid out in a fixed order so each
tile's rows land in ~28 of the group's 56 row-chunks. The pool is written
into a host-side per-core copy of x^T in chunk-load order, so every pool
load is a plain contiguous DMA (no indirect gathers / SWDGE descriptor cost).
PE contracts, per tile, only the chunks that contain that tile's rows with a
host-built scatter matrix (stat). A warmup stream of tiny matmuls ramps the
PE p-state during the initial pool fill; per-tile filler matmuls paper over
the one spot (first tile of group B) where the DMA bus falls behind.

The bass program structure (chunk lists, DMA segments, wait thresholds) is
shared by all 8 cores; only DRAM contents differ. Structure is derived from
`connections` at call time and cached.
"""

import sys

sys.path.insert(0, "/opt/trn_rl_repo")

import hashlib

import numpy as np

import concourse.bass as bass
import concourse.mybir as mybir
from concourse.bass_utils import run_bass_kernel_spmd

B = 1024
IN = 8192
O = 8192
K = 32
NCORES = 8
OC = O // NCORES        # 1024 outputs per core
NG = 2                  # groups per core
GT = 4                  # tiles per group
NT = NG * GT            # 8 tiles per core
SP = 28                 # spare pool chunk slots (= |CL[1][0]|, set by plan)

F16 = mybir.dt.float16
F32 = mybir.dt.float32

# Bucket order for the 15 nonzero 4-bit tile masks, annealed so each tile's
# buckets are nearly contiguous (keeps per-tile chunk counts ~27).
MASK_ORDER = [15, 11, 2, 10, 14, 12, 8, 9, 13, 5, 1, 3, 7, 6, 4]

# PE warmup / filler matmul counts (tiny free=64 matmuls, ~29ns each at full
# clock). W0 spans the initial stat+pool load; FILL[t] covers spots where the
# DMA bus falls behind the PE. Calibrated against TimelineSim.
WARMUP = 800
WARMFREE = 64           # free-dim of warmup/filler matmuls
FILL = [0, 0, 0, 0, 0, 0, 0, 0]

_plan_cache = {}
_stat_cache = {}
_prog_cache = {}


def _digest(*arrs):
    h = hashlib.sha1()
    for a in arrs:
        h.update(a.tobytes())
    return h.hexdigest()


def _make_plan(conn):
    """Shared program structure + per-core row layouts."""
    conn = conn.reshape(NCORES, NG, GT, 128, K)

    # per (core, group): bucket rows by exact tile mask, lay out in MASK_ORDER
    seqs = [[None] * NG for _ in range(NCORES)]   # row ids per slot position
    masks8 = [[None] * NG for _ in range(NCORES)]
    nch = [0] * NG
    for c in range(NCORES):
        for g in range(NG):
            m8 = np.zeros(IN, np.uint8)
            for t in range(GT):
                m8[np.unique(conn[c, g, t])] |= 1 << t
            seq = np.concatenate(
                [np.flatnonzero(m8 == m) for m in MASK_ORDER]
            ).astype(np.int64)
            seqs[c][g] = seq
            masks8[c][g] = m8
            nch[g] = max(nch[g], (len(seq) + 127) // 128)
    NCH = max(nch)

    # cross-core union chunk masks -> shared per-tile chunk lists
    Mc = np.zeros((NG, NCH), np.uint8)
    for c in range(NCORES):
        for g in range(NG):
            seq = seqs[c][g]
            m = np.zeros(NCH * 128, np.uint8)
            m[: len(seq)] = masks8[c][g][seq]
            Mc[g] |= np.bitwise_or.reduce(m.reshape(NCH, 128), axis=1)
    CL = [
        [list(np.flatnonzero((Mc[g] >> t) & 1)) for t in range(GT)]
        for g in range(NG)
    ]

    # pad per-core seqs to NCH*128 with row 0 (stat is zero there)
    for c in range(NCORES):
        for g in range(NG):
            seq = seqs[c][g]
            pad = NCH * 128 - len(seq)
            seqs[c][g] = np.concatenate([seq, np.zeros(pad, np.int64)])

    # chunk load order per group: tile0's chunks, then each tile's extras,
    # then unused chunks. Within each segment chunks used by the group's
    # last tile come last (their slots free latest).
    def load_order(g):
        order, seen, segs = [], set(), []
        last = CL[g][GT - 1]
        for t in range(GT):
            ext = [c for c in CL[g][t] if c not in seen]
            ext.sort(key=lambda c: (c in last, c))
            order += ext
            seen |= set(ext)
            segs.append(len(ext))
        rest = [c for c in range(NCH) if c not in seen]
        order += rest
        segs[-1] += len(rest)
        return order, segs

    orderA, segsA = load_order(0)
    orderB, segsB = load_order(1)

    # slots: group A chunk at load pos p -> slot p. Group B: pos p < SP ->
    # slot NCH+p; later positions reuse A slots, earliest-released first.
    # release class of A slot = last A tile using it (-1 if none).
    sp = len(CL[1][0])
    def aclass(c):
        k = -1
        for t in range(GT):
            if c in CL[0][t]:
                k = t
        return k
    a_release = [aclass(c) for c in orderA]          # per A load pos
    reuse_pos = sorted(range(NCH), key=lambda p: (a_release[p], p))
    slotB = [NCH + p if p < sp else reuse_pos[p - sp] for p in range(NCH)]

    slot_of = [
        {c: p for p, c in enumerate(orderA)},
        {c: slotB[p] for p, c in enumerate(orderB)},
    ]

    # pool DMA segments: (g, lp0, n, dest_slot0, wait)
    # wait: ('none',) | ('pe', thr)  [pe_sem >= thr tiles done]
    dmas = []
    lp = 0
    for t in range(GT):
        n = segsA[t]
        if n:
            dmas.append((0, lp, n, lp, ("none",)))
            lp += n
    nb0 = min(sp, NCH)
    lp = 0
    while lp < nb0:
        n = min(16, nb0 - lp)
        dmas.append((1, lp, n, NCH + lp, ("none",)))
        lp += n
    while lp < NCH:
        # contiguous dest runs within reuse_pos, not crossing a release-class
        # boundary; class k slots are free once tile A_k is done (pe>=k+1),
        # class -1 slots (A chunks used by no tile) are free immediately
        def cls(i):
            return a_release[reuse_pos[i]]
        n = 1
        while (
            lp + n < NCH
            and n < 16
            and reuse_pos[lp + n - sp] == reuse_pos[lp - sp] + n
            and cls(lp + n - sp) == cls(lp - sp)
        ):
            n += 1
        k = cls(lp - sp)
        w = ("none",) if k < 0 else ("pe", k + 1)
        dmas.append((1, lp, n, reuse_pos[lp - sp], w))
        lp += n

    # split oversized A segments (descriptor payload < 64KB -> n <= 31)
    dmas2 = []
    for g, lp0, n, s0, w in dmas:
        while n > 31:
            dmas2.append((g, lp0, 31, s0, w))
            lp0, n, s0 = lp0 + 31, n - 31, s0 + 31
        dmas2.append((g, lp0, n, s0, w))
    dmas = dmas2

    # per-tile pool-DMA requirements: all DMAs covering its chunks' load pos
    posmap = [
        {c: p for p, c in enumerate(orderA)},
        {c: p for p, c in enumerate(orderB)},
    ]
    tile_dma_req = []
    for g in range(NG):
        for t in range(GT):
            need = set()
            maxpos = max(posmap[g][c] for c in CL[g][t])
            for i, (dg, lp0, n, _, _) in enumerate(dmas):
                if dg == g and lp0 <= maxpos:
                    need.add(i)
                elif dg == g and any(
                    lp0 <= posmap[g][c] < lp0 + n for c in CL[g][t]
                ):
                    need.add(i)
            tile_dma_req.append(sorted(need))

    # single bus issue order for all stat + pool DMAs, sorted by the first
    # tile that needs each transfer (stat of a tile before its pool pieces)
    ddl_pool = []
    for i in range(len(dmas)):
        need = [T for T in range(NT) if i in tile_dma_req[T]]
        ddl_pool.append(min(need) if need else NT + 1)
    items = []
    for T in range(NT):
        w = ("pe", T - 3) if T >= 4 else ("none",)
        items.append((T, 0, ("stat", T, w)))
    for i, dma in enumerate(dmas):
        items.append((ddl_pool[i], 1, ("pool", i, dma[4])))
    items.sort(key=lambda e: (e[0], e[1]))
    bus_order = [e[2] for e in items]

    TCMAX = max(len(CL[g][t]) for g in range(NG) for t in range(GT))
    return dict(
        bus_order=bus_order,
        NCH=NCH, CL=CL, seqs=seqs, orderA=orderA, orderB=orderB,
        slot_of=slot_of, dmas=dmas, tile_dma_req=tile_dma_req, TCMAX=TCMAX,
        SP=sp,
    )


def _build_program(plan):
    NCH, TCMAX = plan["NCH"], plan["TCMAX"]
    CL, dmas, req = plan["CL"], plan["dmas"], plan["tile_dma_req"]
    slot_of = plan["slot_of"]
    NPOOL = NCH + SP

    nc = bass.Bass()
    xp_in = nc.declare_dram_parameter("xp", [NG, 128, NCH, B], F16, isOutput=False)
    st_in = nc.declare_dram_parameter("st", [NT, 128, TCMAX * 128], F16, isOutput=False)
    y_out = nc.declare_dram_parameter("y", [NT, 128, B], F16, isOutput=True)

    with (
        nc.sbuf_tensor("pool_sb", [128, NPOOL, B], F16) as pool_sb,
        nc.sbuf_tensor("st_sb", [128, 4, TCMAX * 128], F16) as st_sb,
        nc.sbuf_tensor("out_sb", [128, 4, B], F16) as out_sb,
        nc.Block() as block,
        nc.semaphore("st0") as st0,
        nc.semaphore("st1") as st1,
        nc.semaphore("st2") as st2,
        nc.semaphore("st3") as st3,
        nc.semaphore("pe_sem") as pe_sem,
        nc.semaphore("v_sem") as v_sem,
        nc.semaphore("y0") as y0,
        nc.semaphore("y1") as y1,
        nc.semaphore("y2") as y2,
        nc.semaphore("y3") as y3,
        nc.semaphore("wz") as wz,
    ):
        import contextlib

        with contextlib.ExitStack() as stack:
            psum = [
                stack.enter_context(nc.psum_tensor(f"ps{i}", [128, 512], F32))
                for i in range(8)
            ]
            pool_sems = [
                stack.enter_context(nc.semaphore(f"pd{i}"))
                for i in range(len(dmas))
            ]
            st_sems = [st0, st1, st2, st3]
            y_sems = [y0, y1, y2, y3]

            @block.sync
            def _(sync: bass.BassEngine):
                for kind, i, w in plan["bus_order"]:
                    if w[0] == "pe":
                        sync.wait_ge(pe_sem, w[1])
                    if kind == "stat":
                        g, t = divmod(i, GT)
                        tc = len(CL[g][t])
                        sync.dma_start(
                            out=st_sb[:, i % 4, 0:tc * 128],
                            in_=st_in[i][:, 0:tc * 128],
                        ).then_inc(st_sems[i % 4], 16)
                    else:
                        g, lp0, n, s0, _ = dmas[i]
                        sync.dma_start(
                            out=pool_sb[:, s0:s0 + n, :],
                            in_=xp_in[g, :, lp0:lp0 + n, :],
                        ).then_inc(pool_sems[i], 16)

            @block.tensor
            def _(pe: bass.BassEngine):
                pe.wait_ge(wz, 1)
                for _ in range(WARMUP):
                    pe.matmul(
                        out=psum[0][:, 0:WARMFREE],
                        lhsT=st_sb[:, 0, 0:128],
                        rhs=st_sb[:, 0, 0:WARMFREE],
                        start=True, stop=True,
                    )
                for T in range(NT):
                    g, t = divmod(T, GT)
                    cl = CL[g][t]
                    tc = len(cl)
                    b0 = 2 * (T % 4)
                    if T >= 4:
                        # own psum banks drained (tile T-4) before filler
                        # garbage or real start=True touches them
                        pe.wait_ge(v_sem, T - 3)
                    for _ in range(FILL[T]):
                        pe.matmul(
                            out=psum[b0][:, 0:WARMFREE],
                            lhsT=st_sb[:, T % 4, 0:128],
                            rhs=st_sb[:, T % 4, 0:WARMFREE],
                            start=True, stop=True,
                        )
                    pe.wait_ge(st_sems[T % 4], 16 * (T // 4 + 1))
                    for i in req[T]:
                        pe.wait_ge(pool_sems[i], 16)
                    for bh in range(2):
                        for j, c in enumerate(cl):
                            s = slot_of[g][c]
                            mm = pe.matmul(
                                out=psum[b0 + bh][:],
                                lhsT=st_sb[:, T % 4, j * 128:(j + 1) * 128],
                                rhs=pool_sb[:, s, bh * 512:(bh + 1) * 512],
                                start=(j == 0),
                                stop=(j == tc - 1),
                            )
                    mm.then_inc(pe_sem, 1)

            @block.vector
            def _(vector: bass.BassEngine):
                vector.memset(st_sb[:, 0, 0:256], 0.0).then_inc(wz, 1)
                for T in range(NT):
                    vector.wait_ge(pe_sem, T + 1)
                    if T >= 4:
                        vector.wait_ge(y_sems[T % 4], 16 * (T // 4))
                    b0 = 2 * (T % 4)
                    vector.tensor_copy(
                        out=out_sb[:, T % 4, 0:512], in_=psum[b0][:]
                    )
                    vector.tensor_copy(
                        out=out_sb[:, T % 4, 512:1024], in_=psum[b0 + 1][:]
                    ).then_inc(v_sem, 1)

            @block.scalar
            def _(sc: bass.BassEngine):
                for T in range(NT):
                    # defer y stores off the bus crunch: quad-buffered out_sb
                    # gives 3 tiles of slack
                    sc.wait_ge(v_sem, min(T + 3, NT))
                    sc.dma_start(
                        out=y_out[T], in_=out_sb[:, T % 4, :]
                    ).then_inc(y_sems[T % 4], 16)
                for i in range(4):
                    sc.wait_ge(y_sems[i], 16 * (NT // 4))

    return nc


def _build_inputs(x, conn, weights, plan):
    """Per-core xp (pool chunks in load order) and stat tensors."""
    NCH, TCMAX = plan["NCH"], plan["TCMAX"]
    CL, seqs = plan["CL"], plan["seqs"]
    orders = [plan["orderA"], plan["orderB"]]

    xT16 = np.ascontiguousarray(x.T.astype(np.float16))      # [IN, B]
    conn_r = conn.reshape(NCORES, NG, GT, 128, K)
    w_r = weights.reshape(NCORES, NG, GT, 128, K).astype(np.float32)

    skey = _digest(conn, weights)
    stats = _stat_cache.get(skey)
    build_stat = stats is None
    if build_stat:
        stats = np.zeros((NCORES, NT, 128, TCMAX * 128), np.float16)

    xps = np.empty((NCORES, NG, 128, NCH, B), np.float16)
    for c in range(NCORES):
        for g in range(NG):
            seq = seqs[c][g]                                  # [NCH*128]
            chunk_rows = seq.reshape(NCH, 128)                # content by chunk
            # xp[g, p, lp, :] = xT16[chunk_rows[order[lp], p]]
            rows_lo = chunk_rows[orders[g]]                   # [NCH, 128]
            xps[c, g] = xT16[rows_lo].transpose(1, 0, 2)      # [128, NCH, B]
            if build_stat:
                for t in range(GT):
                    cl = CL[g][t]
                    tc = len(cl)
                    # row -> (j, p) within this tile's chunk list
                    jmap = np.full(IN, -1, np.int32)
                    pmap = np.zeros(IN, np.int32)
                    for j, ch in enumerate(cl):
                        rows_c = chunk_rows[ch]
                        jmap[rows_c] = j
                        pmap[rows_c] = np.arange(128)
                    cb = conn_r[c, g, t]                      # [128, K]
                    wb = w_r[c, g, t]                         # [128, K]
                    j_arr = jmap[cb]
                    p_arr = pmap[cb]
                    assert (j_arr >= 0).all()
                    m_arr = np.broadcast_to(
                        np.arange(128)[:, None], cb.shape
                    )
                    st3 = np.zeros((128, tc, 128), np.float32)
                    np.add.at(st3, (p_arr, j_arr, m_arr), wb)
                    stats[c, GT * g + t, :, 0:tc * 128] = (
                        st3.reshape(128, tc * 128).astype(np.float16)
                    )
    if build_stat:
        _stat_cache.clear()
        _stat_cache[skey] = stats
    return xps, stats


def kernel(x, connections, weights):
    x = np.asarray(x)
    connections = np.asarray(connections)
    weights = np.asarray(weights)

    pkey = _digest(connections)
    plan = _plan_cache.get(pkey)
    if plan is None:
        plan = _make_plan(connections)
        _plan_cache.clear()
        _plan_cache[pkey] = plan

    sig = (plan["NCH"], plan["TCMAX"],
           tuple(tuple(len(c) for c in g) for g in plan["CL"]))
    if sig not in _prog_cache:
        _prog_cache.clear()
        _prog_cache[sig] = _build_program(plan)
    nc = _prog_cache[sig]
    global _cached
    _cached = {sig: nc}

    xps, stats = _build_inputs(x, connections, weights, plan)
    in_maps = [{"xp": xps[c], "st": stats[c]} for c in range(NCORES)]
    res = run_bass_kernel_spmd(nc, in_maps, core_ids=list(range(NCORES)))

    out = np.empty((B, O), np.float32)
    for c in range(NCORES):
        y = res.results[c]["y"]                               # [NT, 128, B] f16
        out[:, c * OC:(c + 1) * OC] = (
            y.reshape(OC, B).T.astype(np.float32)
        )
    return out


_cached = {}


# revision 15
# speedup vs baseline: 1.0388x; 1.0388x over previous
"""PositionalSparseLinear v6: affiliation-bucketed group pools, contiguous
pool DMA, warm PE.

out[b, o] = sum_k x[b, conn[o,k]] * w[o,k]  ==  out = x @ S  (S sparse).

Sharding: out_features across 8 cores (1024 outs each), x replicated.

Per core the 1024 outputs form 2 groups x 4 tiles(128). For each group the
~7100 distinct x-rows its tiles touch are bucketed by the exact 4-bit mask of
which tiles use them, and buckets are laid out in a fixed order so each
tile's rows land in ~28 of the group's 56 row-chunks. The pool is written
into a host-side per-core copy of x^T in chunk-load order, so every pool
load is a plain contiguous DMA (no indirect gathers / SWDGE descriptor cost).
PE contracts, per tile, only the chunks that contain that tile's rows with a
host-built scatter matrix (stat). A warmup stream of tiny matmuls ramps the
PE p-state during the initial pool fill; per-tile filler matmuls paper over
the one spot (first tile of group B) where the DMA bus falls behind.

The bass program structure (chunk lists, DMA segments, wait thresholds) is
shared by all 8 cores; only DRAM contents differ. Structure is derived from
`connections` at call time and cached.
"""

import sys

sys.path.insert(0, "/opt/trn_rl_repo")

import hashlib

import numpy as np

import concourse.bass as bass
import concourse.mybir as mybir
from concourse.bass_utils import run_bass_kernel_spmd

B = 1024
IN = 8192
O = 8192
K = 32
NCORES = 8
OC = O // NCORES        # 1024 outputs per core
NG = 2                  # groups per core
GT = 4                  # tiles per group
NT = NG * GT            # 8 tiles per core
SP = 35                 # spare pool chunk slots (plan may shrink to fit SBUF)

F16 = mybir.dt.float16
F32 = mybir.dt.float32
F8E3 = mybir.dt.float8e3   # 1-3-4: 4 mantissa bits
E3M4_MAX = 15.5            # largest finite e3m4 value

# Bucket order for the 15 nonzero 4-bit tile masks, annealed so each tile's
# buckets are nearly contiguous (keeps per-tile chunk counts ~27).
MASK_ORDER = [15, 11, 2, 10, 14, 12, 8, 9, 13, 5, 1, 3, 7, 6, 4]

# PE warmup / filler matmul counts (tiny free=64 matmuls, ~29ns each at full
# clock). W0 spans the initial stat+pool load; FILL[t] covers spots where the
# DMA bus falls behind the PE. Calibrated against TimelineSim.
WARMUP = 800
WARMFREE = 64           # free-dim of warmup/filler matmuls
STAT_SCALE = 16.0       # power-of-2 lift into e3m4 range; set per dataset
FILL = [0, 0, 0, 0, 0, 0, 0, 0]

_plan_cache = {}
_stat_cache = {}
_prog_cache = {}


def _digest(*arrs):
    h = hashlib.sha1()
    for a in arrs:
        h.update(a.tobytes())
    return h.hexdigest()


def _make_plan(conn):
    """Shared program structure + per-core row layouts."""
    conn = conn.reshape(NCORES, NG, GT, 128, K)

    # per (core, group): bucket rows by exact tile mask, lay out in MASK_ORDER
    seqs = [[None] * NG for _ in range(NCORES)]   # row ids per slot position
    masks8 = [[None] * NG for _ in range(NCORES)]
    nch = [0] * NG
    for c in range(NCORES):
        for g in range(NG):
            m8 = np.zeros(IN, np.uint8)
            for t in range(GT):
                m8[np.unique(conn[c, g, t])] |= 1 << t
            seq = np.concatenate(
                [np.flatnonzero(m8 == m) for m in MASK_ORDER]
            ).astype(np.int64)
            seqs[c][g] = seq
            masks8[c][g] = m8
            nch[g] = max(nch[g], (len(seq) + 127) // 128)
    NCH = max(nch)

    # cross-core union chunk masks -> shared per-tile chunk lists
    Mc = np.zeros((NG, NCH), np.uint8)
    for c in range(NCORES):
        for g in range(NG):
            seq = seqs[c][g]
            m = np.zeros(NCH * 128, np.uint8)
            m[: len(seq)] = masks8[c][g][seq]
            Mc[g] |= np.bitwise_or.reduce(m.reshape(NCH, 128), axis=1)
    CL = [
        [list(np.flatnonzero((Mc[g] >> t) & 1)) for t in range(GT)]
        for g in range(NG)
    ]

    # pad per-core seqs to NCH*128 with row 0 (stat is zero there)
    for c in range(NCORES):
        for g in range(NG):
            seq = seqs[c][g]
            pad = NCH * 128 - len(seq)
            seqs[c][g] = np.concatenate([seq, np.zeros(pad, np.int64)])

    # chunk load order per group: tile0's chunks, then each tile's extras,
    # then unused chunks. Within each segment chunks used by the group's
    # last tile come last (their slots free latest).
    def load_order(g):
        order, seen, segs = [], set(), []
        last = CL[g][GT - 1]
        for t in range(GT):
            ext = [c for c in CL[g][t] if c not in seen]
            ext.sort(key=lambda c: (c in last, c))
            order += ext
            seen |= set(ext)
            segs.append(len(ext))
        rest = [c for c in range(NCH) if c not in seen]
        order += rest
        segs[-1] += len(rest)
        return order, segs

    orderA, segsA = load_order(0)
    orderB, segsB = load_order(1)

    # slots: group A chunk at load pos p -> slot p. Group B: pos p < SP ->
    # slot NCH+p; later positions reuse A slots, earliest-released first.
    # release class of A slot = last A tile using it (-1 if none).
    sp = min(len(CL[1][0]) + 7, 91 - NCH)
    def aclass(c):
        k = -1
        for t in range(GT):
            if c in CL[0][t]:
                k = t
        return k
    a_release = [aclass(c) for c in orderA]          # per A load pos
    reuse_pos = sorted(range(NCH), key=lambda p: (a_release[p], p))
    slotB = [NCH + p if p < sp else reuse_pos[p - sp] for p in range(NCH)]

    slot_of = [
        {c: p for p, c in enumerate(orderA)},
        {c: slotB[p] for p, c in enumerate(orderB)},
    ]

    # pool DMA segments: (g, lp0, n, dest_slot0, wait)
    # wait: ('none',) | ('pe', thr)  [pe_sem >= thr tiles done]
    dmas = []
    lp = 0
    for t in range(GT):
        n = segsA[t]
        if n:
            dmas.append((0, lp, n, lp, ("none",)))
            lp += n
    nb0 = min(sp, NCH)
    lp = 0
    while lp < nb0:
        n = min(16, nb0 - lp)
        dmas.append((1, lp, n, NCH + lp, ("none",)))
        lp += n
    while lp < NCH:
        # contiguous dest runs within reuse_pos, not crossing a release-class
        # boundary; class k slots are free once tile A_k is done (pe>=k+1),
        # class -1 slots (A chunks used by no tile) are free immediately
        def cls(i):
            return a_release[reuse_pos[i]]
        n = 1
        while (
            lp + n < NCH
            and n < 16
            and reuse_pos[lp + n - sp] == reuse_pos[lp - sp] + n
            and cls(lp + n - sp) == cls(lp - sp)
        ):
            n += 1
        k = cls(lp - sp)
        w = ("none",) if k < 0 else ("pe", k + 1)
        dmas.append((1, lp, n, reuse_pos[lp - sp], w))
        lp += n

    # split oversized A segments (descriptor payload < 64KB -> n <= 31)
    dmas2 = []
    for g, lp0, n, s0, w in dmas:
        while n > 31:
            dmas2.append((g, lp0, 31, s0, w))
            lp0, n, s0 = lp0 + 31, n - 31, s0 + 31
        dmas2.append((g, lp0, n, s0, w))
    dmas = dmas2

    # per-tile pool-DMA requirements: all DMAs covering its chunks' load pos
    posmap = [
        {c: p for p, c in enumerate(orderA)},
        {c: p for p, c in enumerate(orderB)},
    ]
    tile_dma_req = []
    for g in range(NG):
        for t in range(GT):
            need = set()
            maxpos = max(posmap[g][c] for c in CL[g][t])
            for i, (dg, lp0, n, _, _) in enumerate(dmas):
                if dg == g and lp0 <= maxpos:
                    need.add(i)
                elif dg == g and any(
                    lp0 <= posmap[g][c] < lp0 + n for c in CL[g][t]
                ):
                    need.add(i)
            tile_dma_req.append(sorted(need))

    # single bus issue order for all stat + pool DMAs, sorted by the first
    # tile that needs each transfer (stat of a tile before its pool pieces)
    ddl_pool = []
    for i in range(len(dmas)):
        need = [T for T in range(NT) if i in tile_dma_req[T]]
        ddl_pool.append(min(need) if need else NT + 1)
    items = []
    for T in range(NT):
        w = ("pe", T - 2) if T >= 3 else ("none",)
        items.append((T, 0, ("stat", T, w)))
    for i, dma in enumerate(dmas):
        items.append((ddl_pool[i], 1, ("pool", i, dma[4])))
    items.sort(key=lambda e: (e[0], e[1]))
    bus_order = [e[2] for e in items]

    TCMAX = max(len(CL[g][t]) for g in range(NG) for t in range(GT))
    return dict(
        bus_order=bus_order,
        NCH=NCH, CL=CL, seqs=seqs, orderA=orderA, orderB=orderB,
        slot_of=slot_of, dmas=dmas, tile_dma_req=tile_dma_req, TCMAX=TCMAX,
        SP=sp,
    )


def _build_program(plan):
    NCH, TCMAX = plan["NCH"], plan["TCMAX"]
    CL, dmas, req = plan["CL"], plan["dmas"], plan["tile_dma_req"]
    slot_of = plan["slot_of"]
    NPOOL = NCH + plan["SP"]

    nc = bass.Bass()
    xp_in = nc.declare_dram_parameter("xp", [NG, 128, NCH, B], F16, isOutput=False)
    st_in = nc.declare_dram_parameter("st", [NT, 128, TCMAX * 128], F8E3, isOutput=False)
    y_out = nc.declare_dram_parameter("y", [NT, 128, B], F16, isOutput=True)

    with (
        nc.sbuf_tensor("pool_sb", [128, NPOOL, B], F16) as pool_sb,
        nc.sbuf_tensor("st_sb", [128, 3, TCMAX * 128], F8E3) as st_sb,
        nc.sbuf_tensor("out_sb", [128, 6, B], F16) as out_sb,
        nc.Block() as block,
        nc.semaphore("st0") as st0,
        nc.semaphore("st1") as st1,
        nc.semaphore("st2") as st2,
        nc.semaphore("pe_sem") as pe_sem,
        nc.semaphore("v_sem") as v_sem,
        nc.semaphore("y0") as y0,
        nc.semaphore("y1") as y1,
        nc.semaphore("y2") as y2,
        nc.semaphore("y3") as y3,
        nc.semaphore("y4") as y4,
        nc.semaphore("y5") as y5,
        nc.semaphore("wz") as wz,
    ):
        import contextlib

        with contextlib.ExitStack() as stack:
            psum = [
                stack.enter_context(nc.psum_tensor(f"ps{i}", [128, 512], F32))
                for i in range(8)
            ]
            pool_sems = [
                stack.enter_context(nc.semaphore(f"pd{i}"))
                for i in range(len(dmas))
            ]
            st_sems = [st0, st1, st2]
            y_sems = [y0, y1, y2, y3, y4, y5]

            @block.sync
            def _(sync: bass.BassEngine):
                for kind, i, w in plan["bus_order"]:
                    if w[0] == "pe":
                        sync.wait_ge(pe_sem, w[1])
                    if kind == "stat":
                        g, t = divmod(i, GT)
                        tc = len(CL[g][t])
                        sync.dma_start(
                            out=st_sb[:, i % 3, 0:tc * 128],
                            in_=st_in[i][:, 0:tc * 128],
                        ).then_inc(st_sems[i % 3], 16)
                    else:
                        g, lp0, n, s0, _ = dmas[i]
                        sync.dma_start(
                            out=pool_sb[:, s0:s0 + n, :],
                            in_=xp_in[g, :, lp0:lp0 + n, :],
                        ).then_inc(pool_sems[i], 16)
                for T in range(NT):
                    sync.wait_ge(v_sem, T + 1)
                    sync.dma_start(
                        out=y_out[T], in_=out_sb[:, T % 6, :]
                    ).then_inc(y_sems[T % 6], 16)
                for i in range(6):
                    sync.wait_ge(y_sems[i], 16 * ((NT - 1 - i) // 6 + 1))

            @block.tensor
            def _(pe: bass.BassEngine):
                pe.wait_ge(wz, 1)
                for _ in range(WARMUP):
                    pe.matmul(
                        out=psum[0][:, 0:WARMFREE],
                        lhsT=st_sb[:, 0, 0:128],
                        rhs=st_sb[:, 0, 0:WARMFREE],
                        start=True, stop=True,
                    )
                for T in range(NT):
                    g, t = divmod(T, GT)
                    cl = CL[g][t]
                    tc = len(cl)
                    b0 = 2 * (T % 4)
                    if T >= 4:
                        # own psum banks drained (tile T-4) before filler
                        # garbage or real start=True touches them
                        pe.wait_ge(v_sem, T - 3)
                    for _ in range(FILL[T]):
                        pe.matmul(
                            out=psum[b0][:, 0:WARMFREE],
                            lhsT=st_sb[:, T % 3, 0:128],
                            rhs=st_sb[:, T % 3, 0:WARMFREE],
                            start=True, stop=True,
                        )
                    pe.wait_ge(st_sems[T % 3], 16 * (T // 3 + 1))
                    for i in req[T]:
                        pe.wait_ge(pool_sems[i], 16)
                    for bh in range(2):
                        for j, c in enumerate(cl):
                            s = slot_of[g][c]
                            mm = pe.matmul(
                                out=psum[b0 + bh][:],
                                lhsT=st_sb[:, T % 3, j * 128:(j + 1) * 128],
                                rhs=pool_sb[:, s, bh * 512:(bh + 1) * 512],
                                start=(j == 0),
                                stop=(j == tc - 1),
                            )
                    mm.then_inc(pe_sem, 1)

            @block.vector
            def _(vector: bass.BassEngine):
                vector.memset(st_sb[:, 0, 0:256], 0.0).then_inc(wz, 1)
                for T in range(NT):
                    vector.wait_ge(pe_sem, T + 1)
                    if T >= 6:
                        vector.wait_ge(y_sems[T % 6], 16 * (T // 6))
                    b0 = 2 * (T % 4)
                    vector.tensor_scalar_mul(
                        out=out_sb[:, T % 6, 0:512], in0=psum[b0][:],
                        scalar1=1.0 / STAT_SCALE,
                    )
                    vector.tensor_scalar_mul(
                        out=out_sb[:, T % 6, 512:1024], in0=psum[b0 + 1][:],
                        scalar1=1.0 / STAT_SCALE,
                    ).then_inc(v_sem, 1)

    return nc


def _build_inputs(x, conn, weights, plan):
    """Per-core xp (pool chunks in load order) and stat tensors."""
    NCH, TCMAX = plan["NCH"], plan["TCMAX"]
    CL, seqs = plan["CL"], plan["seqs"]
    orders = [plan["orderA"], plan["orderB"]]

    xT16 = np.ascontiguousarray(x.T.astype(np.float16))      # [IN, B]
    conn_r = conn.reshape(NCORES, NG, GT, 128, K)
    w_r = weights.reshape(NCORES, NG, GT, 128, K).astype(np.float32)

    import ml_dtypes

    skey = _digest(conn, weights)
    stats = _stat_cache.get(skey)
    build_stat = stats is None
    if build_stat:
        stats = np.zeros((NCORES, NT, 128, TCMAX * 128),
                         ml_dtypes.float8_e3m4)

    raw_stats = []
    xps = np.empty((NCORES, NG, 128, NCH, B), np.float16)
    for c in range(NCORES):
        for g in range(NG):
            seq = seqs[c][g]                                  # [NCH*128]
            chunk_rows = seq.reshape(NCH, 128)                # content by chunk
            # xp[g, p, lp, :] = xT16[chunk_rows[order[lp], p]]
            rows_lo = chunk_rows[orders[g]]                   # [NCH, 128]
            xps[c, g] = xT16[rows_lo].transpose(1, 0, 2)      # [128, NCH, B]
            if build_stat:
                for t in range(GT):
                    cl = CL[g][t]
                    tc = len(cl)
                    # row -> (j, p) within this tile's chunk list
                    jmap = np.full(IN, -1, np.int32)
                    pmap = np.zeros(IN, np.int32)
                    for j, ch in enumerate(cl):
                        rows_c = chunk_rows[ch]
                        jmap[rows_c] = j
                        pmap[rows_c] = np.arange(128)
                    cb = conn_r[c, g, t]                      # [128, K]
                    wb = w_r[c, g, t]                         # [128, K]
                    j_arr = jmap[cb]
                    p_arr = pmap[cb]
                    assert (j_arr >= 0).all()
                    m_arr = np.broadcast_to(
                        np.arange(128)[:, None], cb.shape
                    )
                    st3 = np.zeros((128, tc, 128), np.float32)
                    np.add.at(st3, (p_arr, j_arr, m_arr), wb)
                    raw_stats.append((c, GT * g + t, tc, st3))
    if build_stat:
        gmax = max(float(np.abs(s).max()) for _, _, _, s in raw_stats)
        scale = 2.0 ** int(np.floor(np.log2(E3M4_MAX / gmax)))
        for c, T, tc, st3 in raw_stats:
            stats[c, T, :, 0:tc * 128] = (
                (st3.reshape(128, tc * 128) * scale)
                .astype(ml_dtypes.float8_e3m4)
            )
        _stat_cache.clear()
        _stat_cache[skey] = (stats, scale)
    else:
        stats, scale = stats
    return xps, stats, scale


def kernel(x, connections, weights):
    x = np.asarray(x)
    connections = np.asarray(connections)
    weights = np.asarray(weights)

    pkey = _digest(connections)
    plan = _plan_cache.get(pkey)
    if plan is None:
        plan = _make_plan(connections)
        _plan_cache.clear()
        _plan_cache[pkey] = plan

    xps, stats, scale = _build_inputs(x, connections, weights, plan)
    global STAT_SCALE
    STAT_SCALE = scale
    sig = (plan["NCH"], plan["TCMAX"], scale,
           tuple(tuple(len(c) for c in g) for g in plan["CL"]))
    if sig not in _prog_cache:
        _prog_cache.clear()
        _prog_cache[sig] = _build_program(plan)
    nc = _prog_cache[sig]
    global _cached
    _cached = {sig: nc}
    in_maps = [{"xp": xps[c], "st": stats[c].view(np.uint8)}
               for c in range(NCORES)]
    res = run_bass_kernel_spmd(nc, in_maps, core_ids=list(range(NCORES)))

    out = np.empty((B, O), np.float32)
    for c in range(NCORES):
        y = res.results[c]["y"]                               # [NT, 128, B] f16
        out[:, c * OC:(c + 1) * OC] = (
            y.reshape(OC, B).T.astype(np.float32)
        )
    return out


_cached = {}
